# revision 18
# baseline (speedup 1.0000x reference)
"""JKConv (8-layer GCN + jumping-knowledge max pool) on 8 Trainium2 cores.

Node-partitioned per the sharding hint: core c owns a contiguous block of
nodes. Layers run feature-major (features on the 128 SBUF partitions, nodes
along the free dim):

  z^T   = W^T @ h^T                      (PE matmul, fp16)
  z2    = dinv * z                       (source-side half of GCN norm)
  cc_in = transpose(z2) -> node-major    (PE transpose; dead cols stay 0)
  AllGather(cc_in) -> cc_out [8*NCOL,128]  (halo exchange, fp16)
  msg   = dma_gather(cc_out, src tokens) (transpose-mode SWDGE gather)
  agg   = segment-reduce(msg)            (DVE, per padded-degree class)
  h'    = elu(dinv*agg + b); jk = max(jk, h')

The int16 gather-index limit (32767) forces a 2-way source split: each
destination's in-edges are bucketed by source half (cores 0-3 vs 4-7) and
padded to a multiple of 4 per half. Destinations are sorted by the padded
pair-class (PA, PB) so one template (shared by all 8 cores -> SPMD) gives
fixed-shape reduces; per-core irregularity lives in the gather index data.
Dummy template slots gather from always-zero rows, so they reduce to zero.
"""

import os

os.environ.pop("JAX_PLATFORMS", None)  # need the axon jax platform for PJRT

import sys

if "/opt/trn_rl_repo" not in sys.path:
    sys.path.insert(0, "/opt/trn_rl_repo")

import numpy as np


def _ensure_ntff_hook():
    """The image's antenv lacks axon_hooks; synthesize it so trace=True can
    reach the NTFF profiler via the boot module's ctypes shim."""
    import types

    if "antenv.axon_hooks" in sys.modules:
        return
    mod = types.ModuleType("antenv.axon_hooks")
    mod._hook = None
    mod.set_axon_ntff_profile_hook = lambda h: setattr(mod, "_hook", h)
    mod.get_axon_ntff_profile_hook = lambda: mod._hook
    sys.modules["antenv.axon_hooks"] = mod
    try:
        import antenv

        antenv.axon_hooks = mod
    except ImportError:
        pass
    try:
        from trn_agent_boot.trn_boot import _ntff_profile_via_ctypes

        mod._hook = _ntff_profile_via_ctypes("/opt/axon/libaxon_pjrt.so")
    except Exception:
        pass


N_NODES = 50000
K_LAYERS = 8
N_CORES = 8
D = 128
P = 128

LAST_EXEC_NS = None  # set by kernel() when KERNEL_TRACE=1


# ---------------------------------------------------------------- host side

def _pad4(c):
    return np.maximum(4, 4 * ((c + 3) // 4))


def make_plan(edge_index, n_nodes, n_cores, group_slot_target=16000,
              grid_cap=10016):
    """Graph preprocessing -> SPMD template (plan) + per-core index data.

    The reduce template must be identical on all cores (SPMD). Build it with
    a two-stage "chain balancing": stage 1 assigns every core's dsts into
    common PA buckets (suffix-max over cores => total columns == npc, a dst
    may be promoted into a larger bucket, padding with zero-token slots);
    stage 2 does the same for PB within each PA bucket.
    """
    npc = n_nodes // n_cores
    src = np.concatenate([edge_index[0].astype(np.int64), np.arange(n_nodes)])
    dst = np.concatenate([edge_index[1].astype(np.int64), np.arange(n_nodes)])
    deg = np.bincount(dst, minlength=n_nodes).astype(np.float32)
    dinv = 1.0 / np.sqrt(deg)

    half_split = (n_cores // 2) * npc  # nodes of cores 0..3 are gather-half A
    is_a = src < half_split

    core_of_dst = dst // npc
    dst_local = dst - core_of_dst * npc

    # per (core, local dst): counts per half
    ca = np.zeros((n_cores, npc), np.int64)
    cb = np.zeros((n_cores, npc), np.int64)
    np.add.at(ca, (core_of_dst[is_a], dst_local[is_a]), 1)
    np.add.at(cb, (core_of_dst[~is_a], dst_local[~is_a]), 1)
    pa = _pad4(ca)  # [n_cores, npc]
    pb = _pad4(cb)

    # ---- stage 1: common PA buckets (promotion-up chain balancing)
    pa_vals = np.unique(pa)[::-1]  # descending
    suff = np.zeros((n_cores, len(pa_vals)), np.int64)
    for c in range(n_cores):
        for i, v in enumerate(pa_vals):
            suff[c, i] = (pa[c] >= v).sum()
    T = suff.max(axis=0)
    t_pa = np.diff(np.concatenate([[0], T]))  # bucket sizes, desc pa order

    # per-core: dsts in pa-desc order, positional fill into pa buckets
    stage1 = []  # per core: list over pa buckets of member dst arrays
    for c in range(n_cores):
        order = np.argsort(-pa[c], kind="stable")
        stage1.append(np.split(order, np.cumsum(t_pa)[:-1]))

    # ---- stage 2: PB sub-buckets within each PA bucket
    buckets = []  # (pa, pb, n) template, in build order
    col_of_node = np.full(n_nodes, -1, np.int64)
    ncol_guess = ((npc + 32 + 127) // 128) * 128
    node_of_col = np.full((n_cores, ncol_guess), -1, np.int64)
    col_cursor = 0
    for bi, x in enumerate(pa_vals):
        if t_pa[bi] == 0:
            continue
        members = [stage1[c][bi] for c in range(n_cores)]
        pb_vals = np.unique(
            np.concatenate([pb[c][m] for c, m in enumerate(members)]))[::-1]
        suff2 = np.zeros((n_cores, len(pb_vals)), np.int64)
        for c in range(n_cores):
            mpb = pb[c][members[c]]
            for i, v in enumerate(pb_vals):
                suff2[c, i] = (mpb >= v).sum()
        T2 = suff2.max(axis=0)
        t_pb = np.diff(np.concatenate([[0], T2]))
        for c in range(n_cores):
            m = members[c]
            o = m[np.argsort(-pb[c][m], kind="stable")]
            parts = np.split(o, np.cumsum(t_pb)[:-1])
            cc = col_cursor
            for i in range(len(pb_vals)):
                node_ids = c * npc + parts[i]
                cols = cc + np.arange(len(parts[i]))
                col_of_node[node_ids] = cols
                node_of_col[c, cols] = node_ids
                cc += t_pb[i]
        for i, y in enumerate(pb_vals):
            if t_pb[i] > 0:
                buckets.append((int(x), int(y), int(t_pb[i])))
        col_cursor += int(t_pb.sum())

    assert col_cursor == npc
    ct = npc
    ncol = ncol_guess
    assert 4 * ncol <= 32768, (ncol, "half-A token range exceeds int16")

    # per template column: its bucket dims (PA, PB)
    col_pa = np.concatenate(
        [np.full(n, x, np.int64) for (x, y, n) in buckets])
    col_pb = np.concatenate(
        [np.full(n, y, np.int64) for (x, y, n) in buckets])

    # group the columns so each group's gather fits in SBUF
    groups = []  # (c0, c1, a_off, a_len_pad, b_off, b_len_pad)
    a_off = b_off = 0
    c0 = 0
    acc = 0
    percol = col_pa + col_pb
    acc_a = acc_b = 0
    for j in range(ct + 1):
        if j == ct or (j > c0 and (acc + percol[j] > group_slot_target
                                   or acc_a + col_pa[j] > grid_cap
                                   or acc_b + col_pb[j] > grid_cap)):
            la = int(col_pa[c0:j].sum())
            lb = int(col_pb[c0:j].sum())
            la_pad = ((la + 127) // 128) * 128
            lb_pad = ((lb + 127) // 128) * 128
            groups.append((c0, j, a_off, la_pad, lb_pad, b_off))
            a_off += la_pad
            b_off += lb_pad
            c0 = j
            acc = acc_a = acc_b = 0
        if j < ct:
            acc += percol[j]
            acc_a += col_pa[j]
            acc_b += col_pb[j]
    la_total = a_off
    lb_total = b_off

    # slot base per column in the padded grids
    colbase_a = np.zeros(ct, np.int64)
    colbase_b = np.zeros(ct, np.int64)
    red_a, red_b = [], []  # per group: list of (rel_off, n, P, out_col)
    for (gc0, gc1, aoff, lap, lbp, boff) in groups:
        colbase_a[gc0:gc1] = aoff + np.concatenate(
            [[0], np.cumsum(col_pa[gc0:gc1])])[:-1]
        colbase_b[gc0:gc1] = boff + np.concatenate(
            [[0], np.cumsum(col_pb[gc0:gc1])])[:-1]

        # runs of equal P within the group (A grid)
        ra = []
        r0 = gc0
        for j in range(gc0 + 1, gc1 + 1):
            if j == gc1 or col_pa[j] != col_pa[r0]:
                ra.append((int(colbase_a[r0] - aoff), int(j - r0),
                           int(col_pa[r0]), int(r0)))
                r0 = j
        rb = []
        r0 = gc0
        for j in range(gc0 + 1, gc1 + 1):
            if j == gc1 or col_pb[j] != col_pb[r0]:
                rb.append((int(colbase_b[r0] - boff), int(j - r0),
                           int(col_pb[r0]), int(r0)))
                r0 = j
        red_a.append(ra)
        red_b.append(rb)

    # ---- per-core data ------------------------------------------------
    token = (src // npc) * ncol + col_of_node[src]  # global token per edge

    # zero-token pools (always-dummy cols >= ct in every core block)
    nzc = ncol - ct
    za = (np.arange(la_total, dtype=np.int64) % (4 * nzc))
    zero_a = (za // nzc) * ncol + ct + (za % nzc)
    zb = (np.arange(lb_total, dtype=np.int64) % (4 * nzc))
    zero_b = (zb // nzc) * ncol + ct + (zb % nzc)

    idx_a = np.empty((n_cores, la_total), np.int64)
    idx_b = np.empty((n_cores, lb_total), np.int64)
    dinv_cols = np.zeros((n_cores, ncol), np.float32)
    for c in range(n_cores):
        idx_a[c] = zero_a
        idx_b[c] = zero_b
        sel = core_of_dst == c
        e_dst = dst_local[sel]
        e_tok = token[sel]
        e_isa = is_a[sel]
        e_col = col_of_node[c * npc + e_dst]
        for half, colbase, idx_arr, tok_off in (
            (e_isa, colbase_a, idx_a, 0),
            (~e_isa, colbase_b, idx_b, 4 * ncol),
        ):
            hcol = e_col[half]
            htok = e_tok[half] - tok_off
            o = np.argsort(hcol, kind="stable")
            hcol_s = hcol[o]
            htok_s = htok[o]
            uniq, start_idx, cnt = np.unique(
                hcol_s, return_index=True, return_counts=True)
            ranks = np.arange(len(hcol_s)) - np.repeat(start_idx, cnt)
            slots = colbase[hcol_s] + ranks
            idx_arr[c, slots] = htok_s
        valid = node_of_col[c] >= 0
        dinv_cols[c, valid] = dinv[node_of_col[c, valid]]

    assert idx_a.max() < 32768 and idx_b.max() < 32768
    assert idx_a.min() >= 0 and idx_b.min() >= 0

    def wrap_idx(a):
        # idx i -> [i % 16, i // 16], replicated to 128 partitions
        n = a.shape[0]
        g = a.astype(np.int16).reshape(-1, 16).T  # [16, n//16]
        assert n % 16 == 0
        return np.tile(g, (8, 1))

    plan = dict(
        ncol=ncol, ct=ct, la=la_total, lb=lb_total,
        groups=groups, red_a=red_a, red_b=red_b,
        msg_a_max=max(g[3] for g in groups),
        msg_b_max=max(g[4] for g in groups),
    )
    data = dict(
        idx_a=np.stack([wrap_idx(idx_a[c]) for c in range(n_cores)]),
        idx_b=np.stack([wrap_idx(idx_b[c]) for c in range(n_cores)]),
        dinv_cols=dinv_cols,
        node_of_col=node_of_col,
    )
    return plan, data


# -------------------------------------------------------------- device side

def build(tc, plan, k_layers):
    import concourse.bass as bass  # noqa: F401
    import concourse.mybir as mybir
    from concourse.masks import make_identity

    nc = tc.nc
    f16 = mybir.dt.float16
    f32 = mybir.dt.float32
    i16 = mybir.dt.int16
    NCOL = plan["ncol"]
    LA, LB = plan["la"], plan["lb"]
    K = k_layers
    RG = [list(range(N_CORES))]

    xT = nc.dram_tensor("xT", [P, NCOL], f16, kind="ExternalInput")
    Wall = nc.dram_tensor("Wall", [P, K * D], f16, kind="ExternalInput")
    bias = nc.dram_tensor("bias", [P, K], f16, kind="ExternalInput")
    dinv_d = nc.dram_tensor("dinv", [P, NCOL], f16, kind="ExternalInput")
    idxA_d = nc.dram_tensor("idxA", [P, LA // 16], i16, kind="ExternalInput")
    idxB_d = nc.dram_tensor("idxB", [P, LB // 16], i16, kind="ExternalInput")
    out_d = nc.dram_tensor("out", [NCOL, P], f32, kind="ExternalOutput")

    ntile = NCOL // 128
    NMB = 8  # node-major DMA batch (tiles per DMA)

    with (
        tc.tile_pool(name="persist", bufs=1) as pp,
        tc.tile_pool(name="msg", bufs=2) as msgp,
        tc.tile_pool(name="nm", bufs=2) as nmp,
        tc.tile_pool(name="mmpsum", bufs=2, space="PSUM") as mmp,
        tc.tile_pool(name="tpsum", bufs=4, space="PSUM") as tpp,
        tc.tile_pool(name="dram", bufs=2, space="DRAM") as dramp,
        nc.allow_low_precision(reason="fp16 pipeline, 2e-2 tolerance"),
    ):
        ident = pp.tile([P, P], f16)
        make_identity(nc, ident[:])

        W_s = pp.tile([P, K * D], f16)
        nc.sync.dma_start(out=W_s[:], in_=Wall[:, :])
        bias_s = pp.tile([P, K], f16)
        nc.sync.dma_start(out=bias_s[:], in_=bias[:, :])
        dinv_s = pp.tile([P, NCOL], f16)
        nc.sync.dma_start(out=dinv_s[:], in_=dinv_d[:, :])
        idxA_s = pp.tile([P, LA // 16], i16)
        nc.sync.dma_start(out=idxA_s[:], in_=idxA_d[:, :])
        idxB_s = pp.tile([P, LB // 16], i16)
        nc.sync.dma_start(out=idxB_s[:], in_=idxB_d[:, :])

        h = pp.tile([P, NCOL], f16)
        jk = pp.tile([P, NCOL], f16)
        z2 = pp.tile([P, NCOL], f16)   # also reused as y in the epilogue
        aggA = pp.tile([P, NCOL], f16)  # also t1
        aggB = pp.tile([P, NCOL], f16)  # also the exp(min(y,0))-1 temp

        nc.sync.dma_start(out=h[:], in_=xT[:, :])
        if plan["ct"] < NCOL:  # reduce runs never touch the dead tail
            nc.gpsimd.memset(aggA[:, plan["ct"]:], 0.0)
            nc.gpsimd.memset(aggB[:, plan["ct"]:], 0.0)

        h_in, h_out = h, h
        for l in range(K):
            # z2 = dinv * (W^T @ h)  (feature-major)
            for off in range(0, NCOL, 512):
                w = min(512, NCOL - off)
                pz = mmp.tile([P, 512], f32, tag="pz")
                nc.tensor.matmul(
                    out=pz[:, :w],
                    lhsT=W_s[:, l * D:(l + 1) * D],
                    rhs=h_in[:, off:off + w],
                    start=True, stop=True,
                )
                nc.vector.tensor_mul(
                    out=z2[:, off:off + w], in0=pz[:, :w],
                    in1=dinv_s[:, off:off + w])

            # transpose to node-major, DMA to cc_in, AllGather
            cc_in = dramp.tile([NCOL, P], f16, tag="ccin")
            cc_out = dramp.tile([N_CORES * NCOL, P], f16, tag="ccout",
                                addr_space="Shared")
            for b0 in range(0, ntile, NMB):
                nb = min(NMB, ntile - b0)
                nm = nmp.tile([P, NMB, P], f16, tag="nm")
                for t in range(nb):
                    pt = tpp.tile([P, P], f16, tag="pt")
                    nc.tensor.transpose(
                        out=pt[:], in_=z2[:, (b0 + t) * P:(b0 + t + 1) * P],
                        identity=ident[:])
                    nc.scalar.copy(out=nm[:, t, :], in_=pt[:])
                nc.sync.dma_start(
                    out=cc_in[b0 * P:(b0 + nb) * P, :].rearrange(
                        "(a p) d -> p a d", p=P),
                    in_=nm[:, :nb, :])
            nc.gpsimd.collective_compute(
                "AllGather", mybir.AluOpType.bypass, replica_groups=RG,
                ins=[cc_in[:].opt()], outs=[cc_out[:].opt()])

            # gather + segmented reduce per group
            half = (N_CORES // 2) * NCOL
            for gi, (gc0, gc1, aoff, lap, lbp, boff) in enumerate(
                    plan["groups"]):
                mA = msgp.tile([P, plan["msg_a_max"]], f16, tag="msgA")
                mB = msgp.tile([P, plan["msg_b_max"]], f16, tag="msgB")
                nc.gpsimd.dma_gather(
                    out_ap=mA[:, :lap].rearrange("p (a l) -> p a l", a=1),
                    in_ap=cc_out[0:half, :],
                    idxs_ap=idxA_s[:, aoff // 16:(aoff + lap) // 16],
                    num_idxs=lap, num_idxs_reg=lap, elem_size=D,
                    transpose=True, single_packet=False)
                nc.gpsimd.dma_gather(
                    out_ap=mB[:, :lbp].rearrange("p (a l) -> p a l", a=1),
                    in_ap=cc_out[half:2 * half, :],
                    idxs_ap=idxB_s[:, boff // 16:(boff + lbp) // 16],
                    num_idxs=lbp, num_idxs_reg=lbp, elem_size=D,
                    transpose=True, single_packet=False)
                for (rel, n, pw, outc) in plan["red_a"][gi]:
                    nc.vector.tensor_reduce(
                        out=aggA[:, outc:outc + n],
                        in_=mA[:, rel:rel + n * pw].rearrange(
                            "p (n k) -> p n k", k=pw),
                        axis=mybir.AxisListType.X, op=mybir.AluOpType.add)
                for (rel, n, pw, outc) in plan["red_b"][gi]:
                    nc.vector.tensor_reduce(
                        out=aggB[:, outc:outc + n],
                        in_=mB[:, rel:rel + n * pw].rearrange(
                            "p (n k) -> p n k", k=pw),
                        axis=mybir.AxisListType.X, op=mybir.AluOpType.add)

            # epilogue: y = dinv*(aggA+aggB) + b ; h' = elu(y) ; jk max
            # aliases: aggA also holds t1; z2 holds y; aggB holds the exp tmp
            nc.vector.tensor_add(out=aggA[:], in0=aggA[:], in1=aggB[:])
            nc.vector.tensor_mul(out=aggA[:], in0=aggA[:], in1=dinv_s[:])
            nc.scalar.add(out=z2[:], in_=aggA[:], add=bias_s[:, l:l + 1])
            if l < K - 1:
                nc.vector.tensor_scalar_min(out=aggB[:], in0=z2[:],
                                            scalar1=0.0)
                nc.scalar.activation(
                    out=aggB[:], in_=aggB[:],
                    func=mybir.ActivationFunctionType.Exp)
                nc.vector.tensor_scalar_add(out=aggB[:], in0=aggB[:],
                                            scalar1=-1.0)
                nc.vector.tensor_max(out=h_out[:], in0=z2[:], in1=aggB[:])
            else:
                nc.vector.tensor_copy(out=h_out[:], in_=z2[:])
            if l == 0:
                nc.vector.tensor_copy(out=jk[:], in_=h_out[:])
            else:
                nc.vector.tensor_max(out=jk[:], in0=jk[:], in1=h_out[:])

        # write out: transpose jk back to node-major f32
        for b0 in range(0, ntile, 4):
            nb = min(4, ntile - b0)
            nmf = nmp.tile([P, 4, P], f32, tag="nmf")
            for t in range(nb):
                pt = tpp.tile([P, P], f16, tag="pt")
                nc.tensor.transpose(
                    out=pt[:], in_=jk[:, (b0 + t) * P:(b0 + t + 1) * P],
                    identity=ident[:])
                nc.scalar.copy(out=nmf[:, t, :], in_=pt[:])
            nc.sync.dma_start(
                out=out_d[b0 * P:(b0 + nb) * P, :].rearrange(
                    "(a p) d -> p a d", p=P),
                in_=nmf[:, :nb, :])


# ---------------------------------------------------------------- assembly

def make_inputs(x, W0, b0, Ws, bs, plan, data, n_cores):
    npc = x.shape[0] // n_cores
    ncol = plan["ncol"]
    k = 1 + Ws.shape[0]
    wall = np.concatenate(
        [W0.astype(np.float16)] +
        [Ws[i].astype(np.float16) for i in range(Ws.shape[0])], axis=1)
    ball = np.stack([b0.astype(np.float16)] +
                    [bs[i].astype(np.float16) for i in range(bs.shape[0])],
                    axis=1)
    assert wall.shape == (P, k * D) and ball.shape == (P, k)
    in_maps = []
    for c in range(n_cores):
        xt = np.zeros((P, ncol), np.float16)
        valid = data["node_of_col"][c] >= 0
        xt[:, valid] = x[data["node_of_col"][c][valid]].astype(np.float16).T
        in_maps.append({
            "xT": xt,
            "Wall": wall,
            "bias": ball,
            "dinv": np.broadcast_to(
                data["dinv_cols"][c].astype(np.float16)[None, :],
                (P, ncol)).copy(),
            "idxA": data["idx_a"][c],
            "idxB": data["idx_b"][c],
        })
    return in_maps


def assemble_output(results, data, n_nodes, n_cores):
    out = np.zeros((n_nodes, D), np.float32)
    for c in range(n_cores):
        cols = data["node_of_col"][c]
        valid = cols >= 0
        out[cols[valid]] = results[c]["out"][valid]
    return out


def build_program(plan, k_layers):
    import concourse.bacc as bacc
    import concourse.tile as tile

    nc = bacc.Bacc("TRN2", target_bir_lowering=False, debug=False,
                   num_devices=N_CORES)
    with tile.TileContext(nc) as tc:
        build(tc, plan, k_layers)
    nc.compile()
    return nc


def kernel(x, edge_index, W0, b0, Ws, bs):
    global LAST_EXEC_NS
    x = np.asarray(x, np.float32)
    edge_index = np.asarray(edge_index)

    plan, data = make_plan(edge_index, x.shape[0], N_CORES)
    in_maps = make_inputs(x, np.asarray(W0, np.float32), np.asarray(b0),
                          np.asarray(Ws, np.float32), np.asarray(bs),
                          plan, data, N_CORES)
    nc = build_program(plan, K_LAYERS)

    from concourse.bass_utils import run_bass_kernel_spmd

    trace = os.environ.get("KERNEL_TRACE", "0") == "1"
    if trace:
        _ensure_ntff_hook()
    tmpdir = os.environ.get("KERNEL_TRACE_DIR") or None
    try:
        res = run_bass_kernel_spmd(
            nc, in_maps, core_ids=list(range(N_CORES)), trace=trace,
            tmpdir=tmpdir)
    except Exception:
        if not trace:
            raise
        res = run_bass_kernel_spmd(
            nc, in_maps, core_ids=list(range(N_CORES)), trace=False)
    LAST_EXEC_NS = res.exec_time_ns
    return assemble_output(res.results, data, x.shape[0], N_CORES)
